# revision 31
# baseline (speedup 1.0000x reference)
"""Trainium2 Bass kernel for nn_CpGPredictor (pairwise-token logistic head).

Math: out[b, s] = emb[x[b,s]] . w_prev + emb[x[b,s+1]] . w_curr + bias
With VOCAB=5 the embedding+linear collapses to two 5-entry scalar tables
    p[v] = emb[v] . w_prev,   c[v] = emb[v] . w_curr  (+ bias)
interpolated exactly by quartics:
    p(a) = sgp*Sq(sp*Sq(a+alp) + bp) + rp*a + cp
    c(b) = sgc*Sq(sc*Sq(b+alc2) + bc2) + rc*b + cc

v1 structure (PE-combine): the 5-term combine
    OUT = sgp*YA + sgc*YB + rp*(a+alc2) + rc*(b+alc2) + K'
runs on the otherwise-idle TensorEngine as 4 accumulating matmuls with
diagonal [128,128] weights (built on-device via iota + is_equal during
the input-DMA flight), one PSUM bank per tile.  Elementwise passes:
  WBa = a+alc2, WBb = b+alc2      [TS u8->fp16, 2x_2P]
  SB  = WBb^2                     [TT fp16]
  TB  = sc*SB + bc2; YB = TB^2    [TS 4x + TT]
  GA  = (a+alp)^2                 [tile0: ACT Sq from u8; tile1: TS+TT]
  YA  = Sq(sp*GA + bp)            [ACT]
  OUT = TS(P + K')                [PSUM fp32 -> fp16]
spread across DVE/Pool/ACT so no engine exceeds ~2.2us serial.

Device layout (pure data parallel over batch, 8 NeuronCores):
  - tokens shipped as uint8; each core gets [16, 8193] (last col = pad)
  - each row split into 8 overlapping chunks of 1025 -> 128 partitions
  - input shipped as two overlapping column-halves [128,513] on the two
    HWDGE rings (sync + scalar) so each tile depends on one DMA only
  - PE warmed up with dummy matmuls during the DMA flight (p-state ramp)
  - outputs fp16, DMA'd per sub-tile from sync/scalar as they complete

Self-contained: hardcodes B=128, S=8192, VOCAB=5, 8 cores.
"""

import os
import sys

import numpy as np

for _p in ("/opt/trn_rl_repo", "/root/.axon_site/_ro/trn_rl_repo"):
    if _p not in sys.path and os.path.isdir(_p):
        sys.path.append(_p)

B = 128
S = 8192
VOCAB = 5
EMBED = 128
N_CORES = 8
ROWS = B // N_CORES          # 16 rows per core
CHUNKS = 8                   # chunks per row -> 128 partitions
CHUNK = S // CHUNKS          # 1024 output elements per partition
SPAD = S + 1                 # padded row length
XW = CHUNK + 1               # 1025 tokens per partition
HW0 = 513                    # half0 = cols [0, 513)
T0 = 512                     # tile boundary: tile0 [0,512), tile1 [512,1024)
T1A = 768                    # tile1 output split: [512,768) + [768,1024)
N_WARM = 5                   # PE warmup matmuls

_STATE = {}


def _params(emb_table, lin_w, lin_b):
    """Host-side f64: fold emb+linear+bias into the kernel immediates."""
    emb = np.asarray(emb_table, np.float64)
    lw = np.asarray(lin_w, np.float64).reshape(-1)
    bias = float(np.asarray(lin_b, np.float64).reshape(-1)[0])
    p = emb @ lw[:EMBED]
    c = emb @ lw[EMBED:] + bias

    t = np.arange(VOCAB, dtype=np.float64)
    V = np.vander(t, VOCAB, increasing=True)

    def quartic(vals):
        a = np.linalg.solve(V, vals)
        if abs(a[4]) < 1e-7:
            vals = vals + 1e-6 * np.array([1.0, -4.0, 6.0, -4.0, 1.0])
            a = np.linalg.solve(V, vals)
        return a

    ap = quartic(p)
    alp = ap[3] / (4 * ap[4])
    c0 = ap[2] / (2 * ap[4]) - 2 * alp * alp
    qp = c0 - alp * alp
    rp = ap[1] - 4 * ap[4] * alp * c0
    cp = ap[0] - ap[4] * c0 * c0
    sgp = 1.0 if ap[4] > 0 else -1.0
    sp = np.sqrt(abs(ap[4]))
    bp = qp * sp

    ac = quartic(c)
    alc = ac[3] / (2 * ac[4])
    qc = (ac[2] / ac[4] - alc * alc) / 2
    rc = ac[1] - 2 * ac[4] * alc * qc
    cc = ac[0] - ac[4] * qc * qc
    sgc = 1.0 if ac[4] > 0 else -1.0
    sc = np.sqrt(abs(ac[4]))
    bc = qc * sc

    alc2 = alc / 2
    bc2 = bc - sc * alc2 * alc2    # inner as (b+alc/2)^2; fold -alc^2/4 here
    # PE-combine: linear terms ride on (x + alc2); fold the offsets into K
    Kp = cp + cc - (rp + rc) * alc2
    f = float
    return dict(alp=f(alp), sp=f(sp), bp=f(bp), sgp=f(sgp), rp=f(rp),
                alc2=f(alc2), sc=f(sc), bc2=f(bc2), sgc=f(sgc), rc=f(rc),
                Kp=f(Kp))


def _build_nc(P):
    import concourse.bass as bass
    import concourse.mybir as mybir
    from concourse.ap import AP

    f32 = mybir.dt.float32
    f16 = mybir.dt.float16
    i32 = mybir.dt.int32
    u8 = mybir.dt.uint8
    MUL = mybir.AluOpType.mult
    ADD = mybir.AluOpType.add
    EQ = mybir.AluOpType.is_equal
    SQ = mybir.ActivationFunctionType.Square
    CPY = mybir.ActivationFunctionType.Copy
    IDN = mybir.ActivationFunctionType.Identity

    nc = bass.Bass()
    x_ext = nc.dram_tensor("xin", [ROWS, SPAD], u8, kind="ExternalInput")
    y_ext = nc.dram_tensor("yout", [ROWS, S], f16, kind="ExternalOutput")
    y_dst = y_ext[:, :].rearrange("r (c j) -> (r c) j", j=CHUNK)

    X = nc.alloc_sbuf_tensor("X", [128, HW0], u8)     # cols [0,513)
    X2 = nc.alloc_sbuf_tensor("X2", [128, HW0], u8)   # cols [512,1025)

    W = 512
    WBa0 = nc.alloc_sbuf_tensor("WBa0", [128, W], f16)
    WBb0 = nc.alloc_sbuf_tensor("WBb0", [128, W], f16)
    SB0 = nc.alloc_sbuf_tensor("SB0", [128, W], f16)
    TB0 = nc.alloc_sbuf_tensor("TB0", [128, W], f16)
    YB0 = nc.alloc_sbuf_tensor("YB0", [128, W], f16)
    GA0 = nc.alloc_sbuf_tensor("GA0", [128, W], f16)
    YA0 = nc.alloc_sbuf_tensor("YA0", [128, W], f16)
    WBa1 = nc.alloc_sbuf_tensor("WBa1", [128, W], f16)
    WBb1 = nc.alloc_sbuf_tensor("WBb1", [128, W], f16)
    SB1 = nc.alloc_sbuf_tensor("SB1", [128, W], f16)
    TB1 = nc.alloc_sbuf_tensor("TB1", [128, W], f16)
    YB1 = nc.alloc_sbuf_tensor("YB1", [128, W], f16)
    WAp1 = nc.alloc_sbuf_tensor("WAp1", [128, W], f16)
    GA1 = nc.alloc_sbuf_tensor("GA1", [128, W], f16)
    YA1 = nc.alloc_sbuf_tensor("YA1", [128, W], f16)
    OUT0 = nc.alloc_sbuf_tensor("OUT0", [128, W], f16)
    OUT1 = nc.alloc_sbuf_tensor("OUT1", [128, W], f16)

    IDX = nc.alloc_sbuf_tensor("IDX", [128, 128], i32)
    d_rp = nc.alloc_sbuf_tensor("d_rp", [128, 128], f16)
    d_rc = nc.alloc_sbuf_tensor("d_rc", [128, 128], f16)
    d_sgp = nc.alloc_sbuf_tensor("d_sgp", [128, 128], f16)
    d_sgc = nc.alloc_sbuf_tensor("d_sgc", [128, 128], f16)
    WARM = nc.alloc_sbuf_tensor("WARM", [128, W], f16)
    BIAS = nc.alloc_sbuf_tensor("BIAS", [128, 3], f32)
    DUMMY = nc.alloc_sbuf_tensor("DUMMY", [128, 1], f16)

    P0 = nc.alloc_psum_tensor("P0", [128, W], f32)
    P1 = nc.alloc_psum_tensor("P1", [128, W], f32)
    PW = nc.alloc_psum_tensor("PW", [128, W], f32)

    dsA = nc.alloc_semaphore("dsA")
    dsB = nc.alloc_semaphore("dsB")
    vsem = nc.alloc_semaphore("vsem")
    psem = nc.alloc_semaphore("psem")
    asem = nc.alloc_semaphore("asem")
    tsem = nc.alloc_semaphore("tsem")
    osem = nc.alloc_semaphore("osem")

    # pre-Block: half0 input DMA on the SP HWDGE ring
    srcA = AP(x_ext, 0, [[SPAD, ROWS], [CHUNK, CHUNKS], [1, HW0]])
    nc.sync.dma_start(X[:, :], srcA).then_inc(dsA, 16)
    if os.environ.get("K_INPUT_SPLIT", "1") == "0":
        srcB = AP(x_ext, T0, [[SPAD, ROWS], [CHUNK, CHUNKS], [1, HW0]])
        nc.sync.dma_start(X2[:, :], srcB).then_inc(dsB, 16)

    with nc.Block(no_gpsimd_drain=True) as block:

        @block.sync
        def _(sync):
            # out tile0 [0,512)
            sync.wait_ge(vsem, 11)
            sync.dma_start(y_dst[:, 0:T0], OUT0[:, :]).then_inc(osem, 16)
            _tail = os.environ.get("K_ACT_TAIL", "0")
            if _tail != "0":
                # out tile1b [768,1024)
                sync.wait_ge(asem, 4)
                sync.dma_start(y_dst[:, T1A:CHUNK],
                               OUT1[:, T1A - T0:W]).then_inc(osem, 16)
            else:
                # whole tile1 from sync; OUT1 fully on vector
                sync.wait_ge(vsem, 13)
                sync.dma_start(y_dst[:, T0:CHUNK],
                               OUT1[:, :]).then_inc(osem, 16)
            if _tail == "2":
                # tile1a also from sync
                sync.wait_ge(vsem, 12)
                sync.dma_start(y_dst[:, T0:T1A],
                               OUT1[:, 0:T1A - T0]).then_inc(osem, 16)

        @block.scalar
        def _(scalar):
            # half1 input DMA on the ACT HWDGE ring (cols [512,1025))
            if os.environ.get("K_INPUT_SPLIT", "1") != "0":
                srcB = AP(x_ext, T0, [[SPAD, ROWS], [CHUNK, CHUNKS], [1, HW0]])
                scalar.dma_start(X2[:, :], srcB).then_inc(dsB, 16)
            # table preload for Square during the DMA flight
            const0 = nc.const_aps.tensor(0.0, (128, 1), f32)
            scalar.activation(out=DUMMY[:], in_=const0, func=SQ,
                              bias=0.0, scale=1.0)
            scalar.wait_ge(vsem, 2)
            scalar.wait_ge(dsA, 16)
            scalar.activation(out=GA0[:, :], in_=X[:, 0:T0], func=SQ,
                              bias=BIAS[:, 0:1],
                              scale=1.0).then_inc(asem, 1)
            scalar.wait_ge(asem, 1)
            scalar.activation(out=YA0[:, :], in_=GA0[:, :], func=SQ,
                              bias=BIAS[:, 1:2],
                              scale=P["sp"]).then_inc(asem, 1)   # 2
            scalar.wait_ge(psem, 9)   # GA1
            scalar.activation(out=YA1[:, :], in_=GA1[:, :], func=SQ,
                              bias=BIAS[:, 1:2],
                              scale=P["sp"]).then_inc(asem, 1)   # 3
            _tail = os.environ.get("K_ACT_TAIL", "0")
            if _tail in ("1", "2"):
                # out tile1b compute (PSUM -> fp16 via ACT Copy)
                scalar.wait_ge(tsem, 2)
                scalar.activation(out=OUT1[:, T1A - T0:W],
                                  in_=P1[:, T1A - T0:W], func=IDN,
                                  bias=BIAS[:, 2:3],
                                  scale=1.0).then_inc(asem, 1)  # 4
            if _tail == "1":
                # out tile1a [512,768)
                scalar.wait_ge(vsem, 12)
                scalar.dma_start(y_dst[:, T0:T1A],
                                 OUT1[:, 0:T1A - T0]).then_inc(osem, 16)

        @block.gpsimd
        def _(gpsimd):
            # diag weights during the DMA flight.  NB: gpsimd ops are async
            # within the engine — self-sync via psem between RAW deps.
            gpsimd.iota(IDX[:, :], pattern=[[1, 128]], base=0,
                        channel_multiplier=-1).then_inc(psem, 1)  # 1
            gpsimd.wait_ge(psem, 1)
            for dst, val in ((d_rp, P["rp"]), (d_rc, P["rc"]),
                             (d_sgp, P["sgp"]), (d_sgc, P["sgc"])):
                gpsimd.tensor_scalar(out=dst[:, :], in0=IDX[:, :],
                                     scalar1=0.0, scalar2=val,
                                     op0=EQ, op1=MUL).then_inc(psem, 1)  # 2-5
            # tile1 elementwise (a-side + c-side inner)
            gpsimd.wait_ge(dsB, 16)
            gpsimd.tensor_scalar(out=WBa1[:, :], in0=X2[:, 0:W],
                                 scalar1=1.0, scalar2=P["alc2"],
                                 op0=MUL, op1=ADD).then_inc(psem, 1)  # 6
            gpsimd.tensor_scalar(out=WBb1[:, :], in0=X2[:, 1:HW0],
                                 scalar1=1.0, scalar2=P["alc2"],
                                 op0=MUL, op1=ADD).then_inc(psem, 1)  # 7
            gpsimd.wait_ge(psem, 7)
            gpsimd.tensor_scalar(out=WAp1[:, :], in0=WBa1[:, :],
                                 scalar1=1.0,
                                 scalar2=P["alp"] - P["alc2"],
                                 op0=MUL, op1=ADD).then_inc(psem, 1)  # 8
            gpsimd.wait_ge(psem, 8)
            gpsimd.tensor_tensor(out=GA1[:, :], in0=WAp1[:, :],
                                 in1=WAp1[:, :], op=MUL).then_inc(psem, 1)  # 9
            gpsimd.tensor_tensor(out=SB1[:, :], in0=WBb1[:, :],
                                 in1=WBb1[:, :], op=MUL).then_inc(psem, 1)  # 10


        @block.vector
        def _(vector):
            vector.memset(BIAS[:, 0:1], P["alp"]).then_inc(vsem, 1)
            vector.memset(BIAS[:, 1:2], P["bp"]).then_inc(vsem, 1)
            vector.memset(BIAS[:, 2:3], P["Kp"])
            vector.memset(WARM[:, :], 1.0).then_inc(vsem, 1)
            vector.wait_ge(dsA, 16)
            vector.tensor_scalar(out=WBa0[:, :], in0=X[:, 0:T0],
                                 scalar1=1.0, scalar2=P["alc2"],
                                 op0=MUL, op1=ADD).then_inc(vsem, 1)  # 4
            vector.tensor_scalar(out=WBb0[:, :], in0=X[:, 1:HW0],
                                 scalar1=1.0, scalar2=P["alc2"],
                                 op0=MUL, op1=ADD).then_inc(vsem, 1)  # 5
            vector.wait_ge(vsem, 5)
            vector.tensor_tensor(out=SB0[:, :], in0=WBb0[:, :],
                                 in1=WBb0[:, :], op=MUL).then_inc(vsem, 1)  # 6
            vector.wait_ge(vsem, 6)
            vector.tensor_scalar(out=TB0[:, :], in0=SB0[:, :],
                                 scalar1=P["sc"], scalar2=P["bc2"],
                                 op0=MUL, op1=ADD).then_inc(vsem, 1)  # 7
            vector.wait_ge(vsem, 7)
            vector.tensor_tensor(out=YB0[:, :], in0=TB0[:, :],
                                 in1=TB0[:, :], op=MUL).then_inc(vsem, 1)  # 8
            vector.wait_ge(psem, 10)  # SB1
            vector.tensor_scalar(out=TB1[:, :], in0=SB1[:, :],
                                 scalar1=P["sc"], scalar2=P["bc2"],
                                 op0=MUL, op1=ADD).then_inc(vsem, 1)  # 9
            vector.wait_ge(vsem, 9)
            vector.tensor_tensor(out=YB1[:, :], in0=TB1[:, :],
                                 in1=TB1[:, :], op=MUL).then_inc(vsem, 1)  # 10
            vector.wait_ge(tsem, 1)
            vector.tensor_scalar(out=OUT0[:, :], in0=P0[:, :],
                                 scalar1=1.0, scalar2=P["Kp"],
                                 op0=MUL, op1=ADD).then_inc(vsem, 1)  # 11
            vector.wait_ge(tsem, 2)
            vector.tensor_scalar(out=OUT1[:, 0:T1A - T0],
                                 in0=P1[:, 0:T1A - T0],
                                 scalar1=1.0, scalar2=P["Kp"],
                                 op0=MUL, op1=ADD).then_inc(vsem, 1)  # 12
            if os.environ.get("K_ACT_TAIL", "0") == "0":
                vector.tensor_scalar(out=OUT1[:, T1A - T0:W],
                                     in0=P1[:, T1A - T0:W],
                                     scalar1=1.0, scalar2=P["Kp"],
                                     op0=MUL, op1=ADD).then_inc(vsem, 1)  # 13

        @block.tensor
        def _(tensor):
            # warm up the PE p-state during the DMA flight
            if os.environ.get("K_WARM", "1") != "0":
                tensor.wait_ge(vsem, 3)
                for _ in range(N_WARM):
                    tensor.matmul(PW[:, :], WARM[:, 0:128], WARM[:, :],
                                  start=True, stop=True)
            if os.environ.get("K_PE", "1") == "0":
                tensor.wait_ge(vsem, 8)
                tensor.wait_ge(asem, 2)
                tensor.matmul(P0[:, :], WARM[:, 0:128], WARM[:, :],
                              start=True, stop=True).then_inc(tsem, 1)
                tensor.wait_ge(vsem, 10)
                tensor.wait_ge(asem, 3)
                tensor.matmul(P1[:, :], WARM[:, 0:128], WARM[:, :],
                              start=True, stop=True).then_inc(tsem, 1)
                return
            # tile0 accumulation group
            tensor.wait_ge(vsem, 4)
            tensor.wait_ge(psem, 5)   # diags built
            tensor.matmul(P0[:, :], d_rp[:, :], WBa0[:, :],
                          start=True, stop=False)
            tensor.wait_ge(vsem, 5)
            tensor.matmul(P0[:, :], d_rc[:, :], WBb0[:, :],
                          start=False, stop=False)
            tensor.wait_ge(vsem, 8)
            tensor.matmul(P0[:, :], d_sgc[:, :], YB0[:, :],
                          start=False, stop=False)
            tensor.wait_ge(asem, 2)
            tensor.matmul(P0[:, :], d_sgp[:, :], YA0[:, :],
                          start=False, stop=True).then_inc(tsem, 1)
            # tile1 accumulation group
            tensor.wait_ge(psem, 6)
            tensor.matmul(P1[:, :], d_rp[:, :], WBa1[:, :],
                          start=True, stop=False)
            tensor.wait_ge(psem, 7)
            tensor.matmul(P1[:, :], d_rc[:, :], WBb1[:, :],
                          start=False, stop=False)
            tensor.wait_ge(vsem, 10)
            tensor.matmul(P1[:, :], d_sgc[:, :], YB1[:, :],
                          start=False, stop=False)
            tensor.wait_ge(asem, 3)
            tensor.matmul(P1[:, :], d_sgp[:, :], YA1[:, :],
                          start=False, stop=True).then_inc(tsem, 1)

    return nc


def _get_nc(P):
    key = tuple(sorted(P.items()))
    if _STATE.get("key") != key:
        _STATE["nc"] = _build_nc(P)
        _STATE["key"] = key
    return _STATE["nc"]


def _run(x, emb_table, lin_w, lin_b, trace=False):
    from concourse.bass_utils import run_bass_kernel_spmd

    P = _params(emb_table, lin_w, lin_b)

    xq = np.asarray(x)
    assert xq.shape == (B, S), xq.shape
    xpad = np.zeros((B, SPAD), np.uint8)
    xpad[:, :S] = xq.astype(np.uint8)

    in_maps = [
        {"xin": np.ascontiguousarray(xpad[ROWS * i:ROWS * (i + 1)])}
        for i in range(N_CORES)
    ]
    nc = _get_nc(P)
    res = run_bass_kernel_spmd(nc, in_maps, list(range(N_CORES)), trace=trace)
    y = np.concatenate([res.results[i]["yout"] for i in range(N_CORES)], axis=0)
    return np.ascontiguousarray(y[:, :S - 1]).astype(np.float32), res


def kernel(x, emb_table, lin_w, lin_b):
    y, _ = _run(x, emb_table, lin_w, lin_b, trace=False)
    return y


# revision 34
# speedup vs baseline: 1.3391x; 1.3391x over previous
"""Trainium2 Bass kernel for nn_CpGPredictor (pairwise-token logistic head).

Math: out[b, s] = emb[x[b,s]] . w_prev + emb[x[b,s+1]] . w_curr + bias
With VOCAB=5 the embedding+linear collapses to two 5-entry scalar tables
    p[v] = emb[v] . w_prev,   c[v] = emb[v] . w_curr  (+ bias)
interpolated exactly by quartics:
    p(a) = sgp*Sq(sp*Sq(a+alp) + bp) + rp*a + cp
    c(b) = sgc*Sq(sc*Sq(b+alc2) + bc2) + rc*b + cc

v2 structure: full-width [128,1024] ops; the 5-term combine runs as a
scalar_tensor_tensor (STT) chain - (in0*s)+in1 fused per op:
    R   = rp*a + K''              [DVE TS, u8 even-offset]
    R2  = rc*WB + R               [STT halves on DVE/Pool]
    U   = sgp*YA + R2             [STT halves]
    OUT = sgc*YB + U              [STT halves]
with WB = b+alc2 (Pool, odd-offset u8 ok there), SB = WB^2 (DVE TT),
GA/YA/YB on ACT.  Tails are column-split DVE||Pool; outputs DMA'd per
half from the two HWDGE rings (sync + scalar).

Device layout (pure data parallel over batch, 8 NeuronCores):
  - tokens shipped as uint8; each core gets [16, 8193] (last col = pad)
  - each row split into 8 overlapping chunks of 1025 -> 128 partitions
  - input lands as two column-halves of one X[128,1025] buffer via both
    HWDGE rings concurrently (~1.8us vs 2.8us single-DMA)
  - gpsimd same-engine RAW deps self-synced via psem (Q7s are async)

Self-contained: hardcodes B=128, S=8192, VOCAB=5, 8 cores.
"""

import os
import sys

import numpy as np

for _p in ("/opt/trn_rl_repo", "/root/.axon_site/_ro/trn_rl_repo"):
    if _p not in sys.path and os.path.isdir(_p):
        sys.path.append(_p)

B = 128
S = 8192
VOCAB = 5
EMBED = 128
N_CORES = 8
ROWS = B // N_CORES          # 16 rows per core
CHUNKS = 8                   # chunks per row -> 128 partitions
CHUNK = S // CHUNKS          # 1024 output elements per partition
SPAD = S + 1                 # padded row length
XW = CHUNK + 1               # 1025 tokens per partition
HW0 = 513                    # half0 = cols [0, 513)
TL = 512                     # left/right column split for the tail ops

_STATE = {}


def _params(emb_table, lin_w, lin_b):
    """Host-side f64: fold emb+linear+bias into the kernel immediates."""
    emb = np.asarray(emb_table, np.float64)
    lw = np.asarray(lin_w, np.float64).reshape(-1)
    bias = float(np.asarray(lin_b, np.float64).reshape(-1)[0])
    p = emb @ lw[:EMBED]
    c = emb @ lw[EMBED:] + bias

    t = np.arange(VOCAB, dtype=np.float64)
    V = np.vander(t, VOCAB, increasing=True)

    def quartic(vals):
        a = np.linalg.solve(V, vals)
        if abs(a[4]) < 1e-7:
            vals = vals + 1e-6 * np.array([1.0, -4.0, 6.0, -4.0, 1.0])
            a = np.linalg.solve(V, vals)
        return a

    ap = quartic(p)
    alp = ap[3] / (4 * ap[4])
    c0 = ap[2] / (2 * ap[4]) - 2 * alp * alp
    qp = c0 - alp * alp
    rp = ap[1] - 4 * ap[4] * alp * c0
    cp = ap[0] - ap[4] * c0 * c0
    sgp = 1.0 if ap[4] > 0 else -1.0
    sp = np.sqrt(abs(ap[4]))
    bp = qp * sp

    ac = quartic(c)
    alc = ac[3] / (2 * ac[4])
    qc = (ac[2] / ac[4] - alc * alc) / 2
    rc = ac[1] - 2 * ac[4] * alc * qc
    cc = ac[0] - ac[4] * qc * qc
    sgc = 1.0 if ac[4] > 0 else -1.0
    sc = np.sqrt(abs(ac[4]))
    bc = qc * sc

    alc2 = alc / 2
    bc2 = bc - sc * alc2 * alc2    # inner as (b+alc/2)^2; fold -alc^2/4 here
    # R = rp*a + K2 with K2 folding the constants and rc*alc2 (rc rides WB)
    K2 = cp + cc - rc * alc2
    f = float
    return dict(alp=f(alp), sp=f(sp), bp=f(bp), sgp=f(sgp), rp=f(rp),
                alc2=f(alc2), sc=f(sc), bc2=f(bc2), sgc=f(sgc), rc=f(rc),
                K2=f(K2))


def _build_nc(P):
    import concourse.bass as bass
    import concourse.mybir as mybir
    from concourse.ap import AP

    f32 = mybir.dt.float32
    f16 = mybir.dt.float16
    u8 = mybir.dt.uint8
    MUL = mybir.AluOpType.mult
    ADD = mybir.AluOpType.add
    SUB = mybir.AluOpType.subtract
    SQ = mybir.ActivationFunctionType.Square
    OPP = ADD if P["sgp"] > 0 else SUB   # U = R2 +- YA
    OPC = ADD if P["sgc"] > 0 else SUB   # OUT = U +- YB

    nc = bass.Bass()
    x_ext = nc.dram_tensor("xin", [ROWS, SPAD], u8, kind="ExternalInput")
    y_ext = nc.dram_tensor("yout", [ROWS, S], f16, kind="ExternalOutput")
    y_dst = y_ext[:, :].rearrange("r (c j) -> (r c) j", j=CHUNK)

    X = nc.alloc_sbuf_tensor("X", [128, XW], u8)

    WB = nc.alloc_sbuf_tensor("WB", [128, CHUNK], f16)
    SB = nc.alloc_sbuf_tensor("SB", [128, CHUNK], f16)
    GA = nc.alloc_sbuf_tensor("GA", [128, CHUNK], f16)
    YA = nc.alloc_sbuf_tensor("YA", [128, CHUNK], f16)
    YB = nc.alloc_sbuf_tensor("YB", [128, CHUNK], f16)
    R = nc.alloc_sbuf_tensor("R", [128, CHUNK], f16)
    R2 = nc.alloc_sbuf_tensor("R2", [128, CHUNK], f16)
    U = nc.alloc_sbuf_tensor("U", [128, CHUNK], f16)
    OUT = nc.alloc_sbuf_tensor("OUT", [128, CHUNK], f16)
    BIAS = nc.alloc_sbuf_tensor("BIAS", [128, 3], f32)
    DUMMY = nc.alloc_sbuf_tensor("DUMMY", [128, 1], f16)

    dsA = nc.alloc_semaphore("dsA")
    dsB = nc.alloc_semaphore("dsB")
    vsem = nc.alloc_semaphore("vsem")
    psem = nc.alloc_semaphore("psem")
    asem = nc.alloc_semaphore("asem")
    osem = nc.alloc_semaphore("osem")

    def L(t):
        return t[:, 0:TL]

    def Rt(t):
        return t[:, TL:CHUNK]

    # pre-Block: half0 input DMA on the SP HWDGE ring
    srcA = AP(x_ext, 0, [[SPAD, ROWS], [CHUNK, CHUNKS], [1, HW0]])
    nc.sync.dma_start(X[:, 0:HW0], srcA).then_inc(dsA, 16)

    with nc.Block(no_gpsimd_drain=True) as block:

        @block.sync
        def _(sync):
            # left output half
            sync.wait_ge(vsem, 8)
            sync.dma_start(y_dst[:, 0:TL], L(OUT)).then_inc(osem, 16)

        @block.scalar
        def _(scalar):
            # half1 input DMA on the ACT HWDGE ring (cols [513,1025))
            srcB = AP(x_ext, HW0, [[SPAD, ROWS], [CHUNK, CHUNKS], [1, TL]])
            scalar.dma_start(X[:, HW0:XW], srcB).then_inc(dsB, 16)
            # table preload for Square during the DMA flight
            const0 = nc.const_aps.tensor(0.0, (128, 1), f32)
            scalar.activation(out=DUMMY[:], in_=const0, func=SQ,
                              bias=0.0, scale=1.0)
            scalar.wait_ge(vsem, 3)
            scalar.wait_ge(dsA, 16)
            scalar.wait_ge(dsB, 16)
            scalar.activation(out=GA[:, :], in_=X[:, 0:CHUNK], func=SQ,
                              bias=BIAS[:, 0:1],
                              scale=1.0).then_inc(asem, 1)
            scalar.wait_ge(asem, 1)
            scalar.activation(out=YA[:, :], in_=GA[:, :], func=SQ,
                              bias=BIAS[:, 1:2],
                              scale=P["sp"]).then_inc(asem, 1)
            scalar.wait_ge(vsem, 5)   # SB
            scalar.activation(out=YB[:, :], in_=SB[:, :], func=SQ,
                              bias=BIAS[:, 2:3],
                              scale=P["sc"]).then_inc(asem, 1)
            # right output half
            scalar.wait_ge(psem, 3)
            scalar.dma_start(y_dst[:, TL:CHUNK], Rt(OUT)).then_inc(osem, 16)

        @block.gpsimd
        def _(gpsimd):
            # WB = b + alc2 (odd-offset u8 is fine on Pool)
            gpsimd.wait_ge(dsA, 16)
            gpsimd.wait_ge(dsB, 16)
            gpsimd.tensor_scalar(out=WB[:, :], in0=X[:, 1:XW],
                                 scalar1=1.0, scalar2=P["alc2"],
                                 op0=MUL, op1=ADD).then_inc(psem, 1)  # 1
            # U_right = R2 +- YA   (sgp folded into add/subtract)
            gpsimd.wait_ge(vsem, 6)   # R2 (full, on DVE)
            gpsimd.wait_ge(asem, 2)   # YA
            gpsimd.tensor_tensor(out=Rt(U), in0=Rt(R2), in1=Rt(YA),
                                 op=OPP).then_inc(psem, 1)  # 2
            # OUT_right = U +- YB
            gpsimd.wait_ge(psem, 2)
            gpsimd.wait_ge(asem, 3)   # YB
            gpsimd.tensor_tensor(out=Rt(OUT), in0=Rt(U), in1=Rt(YB),
                                 op=OPC).then_inc(psem, 1)  # 3

        @block.vector
        def _(vector):
            vector.memset(BIAS[:, 0:1], P["alp"]).then_inc(vsem, 1)
            vector.memset(BIAS[:, 1:2], P["bp"]).then_inc(vsem, 1)
            vector.memset(BIAS[:, 2:3], P["bc2"]).then_inc(vsem, 1)
            vector.wait_ge(dsA, 16)
            vector.wait_ge(dsB, 16)
            # R = rp*a + K2 (even-offset u8 fine on DVE)
            vector.tensor_scalar(out=R[:, :], in0=X[:, 0:CHUNK],
                                 scalar1=P["rp"], scalar2=P["K2"],
                                 op0=MUL, op1=ADD).then_inc(vsem, 1)  # 4
            # SB = WB^2
            vector.wait_ge(psem, 1)
            vector.tensor_tensor(out=SB[:, :], in0=WB[:, :],
                                 in1=WB[:, :], op=MUL).then_inc(vsem, 1)  # 5
            # R2 = rc*WB + R (full width; STT is DVE-only)
            vector.wait_ge(vsem, 5)
            vector.scalar_tensor_tensor(out=R2[:, :], in0=WB[:, :],
                                        scalar=P["rc"], in1=R[:, :],
                                        op0=MUL, op1=ADD).then_inc(vsem, 1)  # 6
            # U_left = R2 +- YA
            vector.wait_ge(vsem, 6)
            vector.wait_ge(asem, 2)   # YA
            vector.tensor_tensor(out=L(U), in0=L(R2), in1=L(YA),
                                 op=OPP).then_inc(vsem, 1)  # 7
            # OUT_left = U +- YB
            vector.wait_ge(vsem, 7)
            vector.wait_ge(asem, 3)   # YB
            vector.tensor_tensor(out=L(OUT), in0=L(U), in1=L(YB),
                                 op=OPC).then_inc(vsem, 1)  # 8

    return nc


def _get_nc(P):
    key = tuple(sorted(P.items()))
    if _STATE.get("key") != key:
        _STATE["nc"] = _build_nc(P)
        _STATE["key"] = key
    return _STATE["nc"]


def _run(x, emb_table, lin_w, lin_b, trace=False):
    from concourse.bass_utils import run_bass_kernel_spmd

    P = _params(emb_table, lin_w, lin_b)

    xq = np.asarray(x)
    assert xq.shape == (B, S), xq.shape
    xpad = np.zeros((B, SPAD), np.uint8)
    xpad[:, :S] = xq.astype(np.uint8)

    in_maps = [
        {"xin": np.ascontiguousarray(xpad[ROWS * i:ROWS * (i + 1)])}
        for i in range(N_CORES)
    ]
    nc = _get_nc(P)
    res = run_bass_kernel_spmd(nc, in_maps, list(range(N_CORES)), trace=trace)
    y = np.concatenate([res.results[i]["yout"] for i in range(N_CORES)], axis=0)
    return np.ascontiguousarray(y[:, :S - 1]).astype(np.float32), res


def kernel(x, emb_table, lin_w, lin_b):
    y, _ = _run(x, emb_table, lin_w, lin_b, trace=False)
    return y


# revision 35
# speedup vs baseline: 1.4051x; 1.0492x over previous
"""Trainium2 Bass kernel for nn_CpGPredictor (pairwise-token logistic head).

Math: out[b, s] = emb[x[b,s]] . w_prev + emb[x[b,s+1]] . w_curr + bias
With VOCAB=5 the embedding+linear collapses to two 5-entry scalar tables
    p[v] = emb[v] . w_prev,   c[v] = emb[v] . w_curr  (+ bias)
interpolated exactly by quartics:
    p(a) = sgp*Sq(sp*Sq(a+alp) + bp) + rp*a + cp
    c(b) = sgc*Sq(sc*Sq(b+alc2) + bc2) + rc*b + cc

v2 structure: full-width [128,1024] ops; the 5-term combine runs as a
scalar_tensor_tensor (STT) chain - (in0*s)+in1 fused per op:
    R   = rp*a + K''              [DVE TS, u8 even-offset]
    R2  = rc*WB + R               [STT halves on DVE/Pool]
    U   = sgp*YA + R2             [STT halves]
    OUT = sgc*YB + U              [STT halves]
with WB = b+alc2 (Pool, odd-offset u8 ok there), SB = WB^2 (DVE TT),
GA/YA/YB on ACT.  Tails are column-split DVE||Pool; outputs DMA'd per
half from the two HWDGE rings (sync + scalar).

Device layout (pure data parallel over batch, 8 NeuronCores):
  - tokens shipped as uint8; each core gets [16, 8193] (last col = pad)
  - each row split into 8 overlapping chunks of 1025 -> 128 partitions
  - input lands as two column-halves of one X[128,1025] buffer via both
    HWDGE rings concurrently (~1.8us vs 2.8us single-DMA)
  - gpsimd same-engine RAW deps self-synced via psem (Q7s are async)

Self-contained: hardcodes B=128, S=8192, VOCAB=5, 8 cores.
"""

import os
import sys

import numpy as np

for _p in ("/opt/trn_rl_repo", "/root/.axon_site/_ro/trn_rl_repo"):
    if _p not in sys.path and os.path.isdir(_p):
        sys.path.append(_p)

B = 128
S = 8192
VOCAB = 5
EMBED = 128
N_CORES = 8
ROWS = B // N_CORES          # 16 rows per core
CHUNKS = 8                   # chunks per row -> 128 partitions
CHUNK = S // CHUNKS          # 1024 output elements per partition
SPAD = S + 1                 # padded row length
XW = CHUNK + 1               # 1025 tokens per partition
HW0 = 513                    # half0 = cols [0, 513)
TL = 512                     # left/right column split for the tail ops

_STATE = {}


def _params(emb_table, lin_w, lin_b):
    """Host-side f64: fold emb+linear+bias into the kernel immediates."""
    emb = np.asarray(emb_table, np.float64)
    lw = np.asarray(lin_w, np.float64).reshape(-1)
    bias = float(np.asarray(lin_b, np.float64).reshape(-1)[0])
    p = emb @ lw[:EMBED]
    c = emb @ lw[EMBED:] + bias

    t = np.arange(VOCAB, dtype=np.float64)
    V = np.vander(t, VOCAB, increasing=True)

    def quartic(vals):
        a = np.linalg.solve(V, vals)
        if abs(a[4]) < 1e-7:
            vals = vals + 1e-6 * np.array([1.0, -4.0, 6.0, -4.0, 1.0])
            a = np.linalg.solve(V, vals)
        return a

    ap = quartic(p)
    alp = ap[3] / (4 * ap[4])
    c0 = ap[2] / (2 * ap[4]) - 2 * alp * alp
    qp = c0 - alp * alp
    rp = ap[1] - 4 * ap[4] * alp * c0
    cp = ap[0] - ap[4] * c0 * c0
    sgp = 1.0 if ap[4] > 0 else -1.0
    sp = np.sqrt(abs(ap[4]))
    bp = qp * sp

    ac = quartic(c)
    alc = ac[3] / (2 * ac[4])
    qc = (ac[2] / ac[4] - alc * alc) / 2
    rc = ac[1] - 2 * ac[4] * alc * qc
    cc = ac[0] - ac[4] * qc * qc
    sgc = 1.0 if ac[4] > 0 else -1.0
    sc = np.sqrt(abs(ac[4]))
    bc = qc * sc

    alc2 = alc / 2
    bc2 = bc - sc * alc2 * alc2    # inner as (b+alc/2)^2; fold -alc^2/4 here
    # R = rp*a + K2 with K2 folding the constants and rc*alc2 (rc rides WB)
    K2 = cp + cc - rc * alc2
    f = float
    return dict(alp=f(alp), sp=f(sp), bp=f(bp), sgp=f(sgp), rp=f(rp),
                alc2=f(alc2), sc=f(sc), bc2=f(bc2), sgc=f(sgc), rc=f(rc),
                K2=f(K2))


def _build_nc(P):
    import concourse.bass as bass
    import concourse.mybir as mybir
    from concourse.ap import AP

    f32 = mybir.dt.float32
    f16 = mybir.dt.float16
    u8 = mybir.dt.uint8
    MUL = mybir.AluOpType.mult
    ADD = mybir.AluOpType.add
    SUB = mybir.AluOpType.subtract
    SQ = mybir.ActivationFunctionType.Square
    OPP = ADD if P["sgp"] > 0 else SUB   # U = R2 +- YA
    OPC = ADD if P["sgc"] > 0 else SUB   # OUT = U +- YB

    nc = bass.Bass()
    x_ext = nc.dram_tensor("xin", [ROWS, SPAD], u8, kind="ExternalInput")
    y_ext = nc.dram_tensor("yout", [ROWS, S], f16, kind="ExternalOutput")
    y_dst = y_ext[:, :].rearrange("r (c j) -> (r c) j", j=CHUNK)

    X = nc.alloc_sbuf_tensor("X", [128, XW], u8)

    WB = nc.alloc_sbuf_tensor("WB", [128, CHUNK], f16)
    SB = nc.alloc_sbuf_tensor("SB", [128, CHUNK], f16)
    GA = nc.alloc_sbuf_tensor("GA", [128, CHUNK], f16)
    YA = nc.alloc_sbuf_tensor("YA", [128, CHUNK], f16)
    YB = nc.alloc_sbuf_tensor("YB", [128, CHUNK], f16)
    R = nc.alloc_sbuf_tensor("R", [128, CHUNK], f16)
    R2 = nc.alloc_sbuf_tensor("R2", [128, CHUNK], f16)
    U = nc.alloc_sbuf_tensor("U", [128, CHUNK], f16)
    OUT = nc.alloc_sbuf_tensor("OUT", [128, CHUNK], f16)
    BIAS = nc.alloc_sbuf_tensor("BIAS", [128, 3], f32)
    DUMMY = nc.alloc_sbuf_tensor("DUMMY", [128, 1], f16)

    dsA = nc.alloc_semaphore("dsA")
    dsB = nc.alloc_semaphore("dsB")
    vsem = nc.alloc_semaphore("vsem")
    psem = nc.alloc_semaphore("psem")
    asem = nc.alloc_semaphore("asem")
    osem = nc.alloc_semaphore("osem")

    def L(t):
        return t[:, 0:TL]

    def Rt(t):
        return t[:, TL:CHUNK]

    # pre-Block: half0 input DMA on the SP HWDGE ring
    srcA = AP(x_ext, 0, [[SPAD, ROWS], [CHUNK, CHUNKS], [1, HW0]])
    nc.sync.dma_start(X[:, 0:HW0], srcA).then_inc(dsA, 16)

    with nc.Block(no_gpsimd_drain=True) as block:

        @block.sync
        def _(sync):
            # left output half
            sync.wait_ge(vsem, 8)
            sync.dma_start(y_dst[:, 0:TL], L(OUT)).then_inc(osem, 16)

        @block.scalar
        def _(scalar):
            # half1 input DMA on the ACT HWDGE ring (cols [513,1025))
            srcB = AP(x_ext, HW0, [[SPAD, ROWS], [CHUNK, CHUNKS], [1, TL]])
            scalar.dma_start(X[:, HW0:XW], srcB).then_inc(dsB, 16)
            # table preload for Square during the DMA flight
            const0 = nc.const_aps.tensor(0.0, (128, 1), f32)
            scalar.activation(out=DUMMY[:], in_=const0, func=SQ,
                              bias=0.0, scale=1.0)
            scalar.wait_ge(vsem, 3)
            scalar.wait_ge(dsA, 16)
            scalar.wait_ge(dsB, 16)
            scalar.activation(out=GA[:, :], in_=X[:, 0:CHUNK], func=SQ,
                              bias=BIAS[:, 0:1],
                              scale=1.0).then_inc(asem, 1)
            scalar.wait_ge(asem, 1)
            scalar.activation(out=YA[:, :], in_=GA[:, :], func=SQ,
                              bias=BIAS[:, 1:2],
                              scale=P["sp"]).then_inc(asem, 1)
            scalar.wait_ge(vsem, 5)   # SB
            scalar.activation(out=YB[:, :], in_=SB[:, :], func=SQ,
                              bias=BIAS[:, 2:3],
                              scale=P["sc"]).then_inc(asem, 1)
            # right output half
            scalar.wait_ge(vsem, 9)
            scalar.dma_start(y_dst[:, TL:CHUNK], Rt(OUT)).then_inc(osem, 16)

        @block.gpsimd
        def _(gpsimd):
            # WB = b + alc2, two halves (odd-offset u8 is fine on Pool)
            gpsimd.wait_ge(dsA, 16)
            gpsimd.tensor_scalar(out=L(WB), in0=X[:, 1:TL + 1],
                                 scalar1=1.0, scalar2=P["alc2"],
                                 op0=MUL, op1=ADD).then_inc(psem, 1)  # 1
            gpsimd.wait_ge(dsB, 16)
            gpsimd.tensor_scalar(out=Rt(WB), in0=X[:, TL + 1:XW],
                                 scalar1=1.0, scalar2=P["alc2"],
                                 op0=MUL, op1=ADD).then_inc(psem, 1)  # 2

        @block.vector
        def _(vector):
            vector.memset(BIAS[:, 0:1], P["alp"]).then_inc(vsem, 1)
            vector.memset(BIAS[:, 1:2], P["bp"]).then_inc(vsem, 1)
            vector.memset(BIAS[:, 2:3], P["bc2"]).then_inc(vsem, 1)
            vector.wait_ge(dsA, 16)
            vector.wait_ge(dsB, 16)
            # R = rp*a + K2 (even-offset u8 fine on DVE)
            vector.tensor_scalar(out=R[:, :], in0=X[:, 0:CHUNK],
                                 scalar1=P["rp"], scalar2=P["K2"],
                                 op0=MUL, op1=ADD).then_inc(vsem, 1)  # 4
            # SB = WB^2, halves behind the WB halves
            vector.wait_ge(psem, 1)
            vector.tensor_tensor(out=L(SB), in0=L(WB),
                                 in1=L(WB), op=MUL)
            vector.wait_ge(psem, 2)
            vector.tensor_tensor(out=Rt(SB), in0=Rt(WB),
                                 in1=Rt(WB), op=MUL).then_inc(vsem, 1)  # 5
            # R2 = rc*WB + R (full width; STT is DVE-only)
            vector.wait_ge(vsem, 5)
            vector.scalar_tensor_tensor(out=R2[:, :], in0=WB[:, :],
                                        scalar=P["rc"], in1=R[:, :],
                                        op0=MUL, op1=ADD).then_inc(vsem, 1)  # 6
            # U = sgp*YA + R2 (STT full; TT-subtract has no 2x uop)
            vector.wait_ge(vsem, 6)
            vector.wait_ge(asem, 2)   # YA
            vector.scalar_tensor_tensor(out=U[:, :], in0=YA[:, :],
                                        scalar=P["sgp"], in1=R2[:, :],
                                        op0=MUL, op1=ADD).then_inc(vsem, 1)  # 7
            # OUT halves = U +- YB (ADD runs 2x; for sgc<0 fold via U)
            vector.wait_ge(vsem, 7)
            vector.wait_ge(asem, 3)   # YB
            vector.tensor_tensor(out=L(OUT), in0=L(U), in1=L(YB),
                                 op=OPC).then_inc(vsem, 1)  # 8
            vector.wait_ge(vsem, 8)
            vector.tensor_tensor(out=Rt(OUT), in0=Rt(U), in1=Rt(YB),
                                 op=OPC).then_inc(vsem, 1)  # 9

    return nc


def _get_nc(P):
    key = tuple(sorted(P.items()))
    if _STATE.get("key") != key:
        _STATE["nc"] = _build_nc(P)
        _STATE["key"] = key
    return _STATE["nc"]


def _run(x, emb_table, lin_w, lin_b, trace=False):
    from concourse.bass_utils import run_bass_kernel_spmd

    P = _params(emb_table, lin_w, lin_b)

    xq = np.asarray(x)
    assert xq.shape == (B, S), xq.shape
    xpad = np.zeros((B, SPAD), np.uint8)
    xpad[:, :S] = xq.astype(np.uint8)

    in_maps = [
        {"xin": np.ascontiguousarray(xpad[ROWS * i:ROWS * (i + 1)])}
        for i in range(N_CORES)
    ]
    nc = _get_nc(P)
    res = run_bass_kernel_spmd(nc, in_maps, list(range(N_CORES)), trace=trace)
    y = np.concatenate([res.results[i]["yout"] for i in range(N_CORES)], axis=0)
    return np.ascontiguousarray(y[:, :S - 1]).astype(np.float32), res


def kernel(x, emb_table, lin_w, lin_b):
    y, _ = _run(x, emb_table, lin_w, lin_b, trace=False)
    return y


# revision 36
# speedup vs baseline: 1.4463x; 1.0293x over previous
"""Trainium2 Bass kernel for nn_CpGPredictor (pairwise-token logistic head).

Math: out[b, s] = emb[x[b,s]] . w_prev + emb[x[b,s+1]] . w_curr + bias
With VOCAB=5 the embedding+linear collapses to two 5-entry scalar tables
    p[v] = emb[v] . w_prev,   c[v] = emb[v] . w_curr  (+ bias)
interpolated exactly by quartics:
    p(a) = sgp*Sq(sp*Sq(a+alp) + bp) + rp*a + cp
    c(b) = sgc*Sq(sc*Sq(b+alc2) + bc2) + rc*b + cc

v2 structure: full-width [128,1024] ops; the 5-term combine runs as a
scalar_tensor_tensor (STT) chain - (in0*s)+in1 fused per op:
    R   = rp*a + K''              [DVE TS, u8 even-offset]
    R2  = rc*WB + R               [STT halves on DVE/Pool]
    U   = sgp*YA + R2             [STT halves]
    OUT = sgc*YB + U              [STT halves]
with WB = b+alc2 (Pool, odd-offset u8 ok there), SB = WB^2 (DVE TT),
GA/YA/YB on ACT.  Tails are column-split DVE||Pool; outputs DMA'd per
half from the two HWDGE rings (sync + scalar).

Device layout (pure data parallel over batch, 8 NeuronCores):
  - tokens shipped as uint8; each core gets [16, 8193] (last col = pad)
  - each row split into 8 overlapping chunks of 1025 -> 128 partitions
  - input lands as two column-halves of one X[128,1025] buffer via both
    HWDGE rings concurrently (~1.8us vs 2.8us single-DMA)
  - gpsimd same-engine RAW deps self-synced via psem (Q7s are async)

Self-contained: hardcodes B=128, S=8192, VOCAB=5, 8 cores.
"""

import os
import sys

import numpy as np

for _p in ("/opt/trn_rl_repo", "/root/.axon_site/_ro/trn_rl_repo"):
    if _p not in sys.path and os.path.isdir(_p):
        sys.path.append(_p)

B = 128
S = 8192
VOCAB = 5
EMBED = 128
N_CORES = 8
ROWS = B // N_CORES          # 16 rows per core
CHUNKS = 8                   # chunks per row -> 128 partitions
CHUNK = S // CHUNKS          # 1024 output elements per partition
SPAD = S + 1                 # padded row length
XW = CHUNK + 1               # 1025 tokens per partition
HW0 = 513                    # half0 = cols [0, 513)
TL = 512                     # left/right column split for the tail ops

_STATE = {}


def _params(emb_table, lin_w, lin_b):
    """Host-side f64: fold emb+linear+bias into the kernel immediates."""
    emb = np.asarray(emb_table, np.float64)
    lw = np.asarray(lin_w, np.float64).reshape(-1)
    bias = float(np.asarray(lin_b, np.float64).reshape(-1)[0])
    p = emb @ lw[:EMBED]
    c = emb @ lw[EMBED:] + bias

    t = np.arange(VOCAB, dtype=np.float64)
    V = np.vander(t, VOCAB, increasing=True)

    def quartic(vals):
        a = np.linalg.solve(V, vals)
        if abs(a[4]) < 1e-7:
            vals = vals + 1e-6 * np.array([1.0, -4.0, 6.0, -4.0, 1.0])
            a = np.linalg.solve(V, vals)
        return a

    ap = quartic(p)
    alp = ap[3] / (4 * ap[4])
    c0 = ap[2] / (2 * ap[4]) - 2 * alp * alp
    qp = c0 - alp * alp
    rp = ap[1] - 4 * ap[4] * alp * c0
    cp = ap[0] - ap[4] * c0 * c0
    sgp = 1.0 if ap[4] > 0 else -1.0
    sp = np.sqrt(abs(ap[4]))
    bp = qp * sp

    ac = quartic(c)
    alc = ac[3] / (2 * ac[4])
    qc = (ac[2] / ac[4] - alc * alc) / 2
    rc = ac[1] - 2 * ac[4] * alc * qc
    cc = ac[0] - ac[4] * qc * qc
    sgc = 1.0 if ac[4] > 0 else -1.0
    sc = np.sqrt(abs(ac[4]))
    bc = qc * sc

    alc2 = alc / 2
    bc2 = bc - sc * alc2 * alc2    # inner as (b+alc/2)^2; fold -alc^2/4 here
    # R = rp*a + K2 with K2 folding the constants and rc*alc2 (rc rides WB)
    K2 = cp + cc - rc * alc2
    f = float
    return dict(alp=f(alp), sp=f(sp), bp=f(bp), sgp=f(sgp), rp=f(rp),
                alc2=f(alc2), sc=f(sc), bc2=f(bc2), sgc=f(sgc), rc=f(rc),
                K2=f(K2))


def _build_nc(P):
    import concourse.bass as bass
    import concourse.mybir as mybir
    from concourse.ap import AP

    f32 = mybir.dt.float32
    f16 = mybir.dt.float16
    u8 = mybir.dt.uint8
    MUL = mybir.AluOpType.mult
    ADD = mybir.AluOpType.add
    SUB = mybir.AluOpType.subtract
    SQ = mybir.ActivationFunctionType.Square
    OPP = ADD if P["sgp"] > 0 else SUB   # U = R2 +- YA
    OPC = ADD if P["sgc"] > 0 else SUB   # OUT = U +- YB

    nc = bass.Bass()
    x_ext = nc.dram_tensor("xin", [ROWS, SPAD], u8, kind="ExternalInput")
    y_ext = nc.dram_tensor("yout", [ROWS, S], f16, kind="ExternalOutput")
    y_dst = y_ext[:, :].rearrange("r (c j) -> (r c) j", j=CHUNK)

    X = nc.alloc_sbuf_tensor("X", [128, XW], u8)

    WB = nc.alloc_sbuf_tensor("WB", [128, CHUNK], f16)
    SB = nc.alloc_sbuf_tensor("SB", [128, CHUNK], f16)
    GA = nc.alloc_sbuf_tensor("GA", [128, CHUNK], f16)
    YA = nc.alloc_sbuf_tensor("YA", [128, CHUNK], f16)
    YB = nc.alloc_sbuf_tensor("YB", [128, CHUNK], f16)
    R = nc.alloc_sbuf_tensor("R", [128, CHUNK], f16)
    R2 = nc.alloc_sbuf_tensor("R2", [128, CHUNK], f16)
    U = nc.alloc_sbuf_tensor("U", [128, CHUNK], f16)
    OUT = nc.alloc_sbuf_tensor("OUT", [128, CHUNK], f16)
    BIAS = nc.alloc_sbuf_tensor("BIAS", [128, 3], f32)
    DUMMY = nc.alloc_sbuf_tensor("DUMMY", [128, 1], f16)

    dsA = nc.alloc_semaphore("dsA")
    dsB = nc.alloc_semaphore("dsB")
    vsem = nc.alloc_semaphore("vsem")
    psem = nc.alloc_semaphore("psem")
    asem = nc.alloc_semaphore("asem")
    osem = nc.alloc_semaphore("osem")

    def L(t):
        return t[:, 0:TL]

    def Rt(t):
        return t[:, TL:CHUNK]

    # pre-Block: half0 input DMA on the SP HWDGE ring
    srcA = AP(x_ext, 0, [[SPAD, ROWS], [CHUNK, CHUNKS], [1, HW0]])
    nc.sync.dma_start(X[:, 0:HW0], srcA).then_inc(dsA, 16)

    with nc.Block(no_gpsimd_drain=True) as block:

        @block.sync
        def _(sync):
            # left output half
            sync.wait_ge(vsem, 8)
            sync.dma_start(y_dst[:, 0:TL], L(OUT)).then_inc(osem, 16)

        @block.scalar
        def _(scalar):
            # half1 input DMA on the ACT HWDGE ring (cols [513,1025))
            srcB = AP(x_ext, HW0, [[SPAD, ROWS], [CHUNK, CHUNKS], [1, TL]])
            scalar.dma_start(X[:, HW0:XW], srcB).then_inc(dsB, 16)
            # table preload for Square during the DMA flight
            const0 = nc.const_aps.tensor(0.0, (128, 1), f32)
            scalar.activation(out=DUMMY[:], in_=const0, func=SQ,
                              bias=0.0, scale=1.0)
            scalar.wait_ge(vsem, 3)
            scalar.wait_ge(dsA, 16)
            scalar.wait_ge(dsB, 16)
            scalar.activation(out=GA[:, :], in_=X[:, 0:CHUNK], func=SQ,
                              bias=BIAS[:, 0:1],
                              scale=1.0).then_inc(asem, 1)
            scalar.wait_ge(asem, 1)
            scalar.activation(out=YA[:, :], in_=GA[:, :], func=SQ,
                              bias=BIAS[:, 1:2],
                              scale=P["sp"]).then_inc(asem, 1)
            scalar.wait_ge(vsem, 6)   # SB
            scalar.activation(out=YB[:, :], in_=SB[:, :], func=SQ,
                              bias=BIAS[:, 2:3],
                              scale=P["sc"]).then_inc(asem, 1)
            # right output half
            scalar.wait_ge(vsem, 9)
            scalar.dma_start(y_dst[:, TL:CHUNK], Rt(OUT)).then_inc(osem, 16)

        @block.gpsimd
        def _(gpsimd):
            # WB = b + alc2, two halves (odd-offset u8 is fine on Pool)
            gpsimd.wait_ge(dsA, 16)
            gpsimd.tensor_scalar(out=L(WB), in0=X[:, 1:TL + 1],
                                 scalar1=1.0, scalar2=P["alc2"],
                                 op0=MUL, op1=ADD).then_inc(psem, 1)  # 1
            gpsimd.wait_ge(dsB, 16)
            gpsimd.tensor_scalar(out=Rt(WB), in0=X[:, TL + 1:XW],
                                 scalar1=1.0, scalar2=P["alc2"],
                                 op0=MUL, op1=ADD).then_inc(psem, 1)  # 2

        @block.vector
        def _(vector):
            vector.memset(BIAS[:, 0:1], P["alp"]).then_inc(vsem, 1)
            vector.memset(BIAS[:, 1:2], P["bp"]).then_inc(vsem, 1)
            vector.memset(BIAS[:, 2:3], P["bc2"]).then_inc(vsem, 1)
            vector.wait_ge(dsA, 16)
            vector.wait_ge(dsB, 16)
            # R = rp*a + K2 (even-offset u8 fine on DVE)
            vector.tensor_scalar(out=R[:, :], in0=X[:, 0:CHUNK],
                                 scalar1=P["rp"], scalar2=P["K2"],
                                 op0=MUL, op1=ADD).then_inc(vsem, 1)  # 4
            # SB_left behind WB_left
            vector.wait_ge(psem, 1)
            vector.tensor_tensor(out=L(SB), in0=L(WB),
                                 in1=L(WB), op=MUL)
            # R2 = rc*WB + R as soon as WB_right lands (before SB_right:
            # it gates the U chain; SB_right only gates YB via ACT queue)
            vector.wait_ge(psem, 2)
            vector.wait_ge(vsem, 4)   # self: R
            vector.scalar_tensor_tensor(out=R2[:, :], in0=WB[:, :],
                                        scalar=P["rc"], in1=R[:, :],
                                        op0=MUL, op1=ADD).then_inc(vsem, 1)  # 5
            vector.tensor_tensor(out=Rt(SB), in0=Rt(WB),
                                 in1=Rt(WB), op=MUL).then_inc(vsem, 1)  # 6
            # U = sgp*YA + R2 (STT full; TT-subtract has no 2x uop)
            vector.wait_ge(vsem, 5)   # self: R2
            vector.wait_ge(asem, 2)   # YA
            vector.scalar_tensor_tensor(out=U[:, :], in0=YA[:, :],
                                        scalar=P["sgp"], in1=R2[:, :],
                                        op0=MUL, op1=ADD).then_inc(vsem, 1)  # 7
            # OUT halves = U +- YB
            vector.wait_ge(vsem, 7)   # self: U
            vector.wait_ge(asem, 3)   # YB
            vector.tensor_tensor(out=L(OUT), in0=L(U), in1=L(YB),
                                 op=OPC).then_inc(vsem, 1)  # 8
            vector.wait_ge(vsem, 8)
            vector.tensor_tensor(out=Rt(OUT), in0=Rt(U), in1=Rt(YB),
                                 op=OPC).then_inc(vsem, 1)  # 9

    return nc


def _get_nc(P):
    key = tuple(sorted(P.items()))
    if _STATE.get("key") != key:
        _STATE["nc"] = _build_nc(P)
        _STATE["key"] = key
    return _STATE["nc"]


def _run(x, emb_table, lin_w, lin_b, trace=False):
    from concourse.bass_utils import run_bass_kernel_spmd

    P = _params(emb_table, lin_w, lin_b)

    xq = np.asarray(x)
    assert xq.shape == (B, S), xq.shape
    xpad = np.zeros((B, SPAD), np.uint8)
    xpad[:, :S] = xq.astype(np.uint8)

    in_maps = [
        {"xin": np.ascontiguousarray(xpad[ROWS * i:ROWS * (i + 1)])}
        for i in range(N_CORES)
    ]
    nc = _get_nc(P)
    res = run_bass_kernel_spmd(nc, in_maps, list(range(N_CORES)), trace=trace)
    y = np.concatenate([res.results[i]["yout"] for i in range(N_CORES)], axis=0)
    return np.ascontiguousarray(y[:, :S - 1]).astype(np.float32), res


def kernel(x, emb_table, lin_w, lin_b):
    y, _ = _run(x, emb_table, lin_w, lin_b, trace=False)
    return y
